# revision 4
# baseline (speedup 1.0000x reference)
"""Grouped MoE dispatcher kernel for 8 Trainium2 NeuronCores.

Expert-parallel: 8 experts per core. Host performs the dispatch (stable sort
of (token, slot) assignments by expert id — identical to the reference's
fixed-capacity grouped dispatch) and supplies each core its 8 experts'
tokens pre-gathered and pre-tiled into SBUF layout; the device runs the
grouped FFN (x@W1 -> silu -> @W2, scaled by routing weight) as bf16 matmuls
with fp32 PSUM accumulation; host scatter-combines the two slots per token.

All DRAM tensors are laid out so every DMA descriptor is one full SBUF
partition line (1-8 KB contiguous), and expert 0's x/W1 are additionally
split into fine-grained chunks so the first matmul starts as soon as
~256 KB have landed. y is stored bf16 (host upcasts when combining).

Problem constants (hardcoded): B=16384 tokens, K=2, E=64 experts, H=512,
F=1024; I/O fp32, matmul operands bf16 (end-to-end rel err ~3.4e-3).
"""

import json
import os

import ml_dtypes
import numpy as np

import concourse.bass as bass
import concourse.bass2jax as bass2jax
import concourse.bass_utils as bass_utils
import concourse.mybir as mybir
import concourse.tile as tile_mod
from concourse.tile import TileContext, ScopedClock
from concourse.bass_utils import run_bass_kernel_spmd

B = 16384
K = 2
E = 64
H = 512
F = 1024
NCORES = 8
EPC = E // NCORES          # experts per core = 8
N = B * K                  # assignments = 32768
CAP = N // E               # per-expert capacity = 512
TPC = EPC * CAP            # tokens (assignments) per core = 4096
P = 128                    # partitions

FP32 = mybir.dt.float32
BF16 = mybir.dt.bfloat16

HS = H // P   # 4 contraction subtiles for stage 1
FS = F // P   # 8 F subtiles (stage-1 out partitions / stage-2 contraction)
CS = CAP // P  # 4 token subtiles per expert


# ---------------------------------------------------------------------------
# Workaround: the walrus build in this container rejects instructions carrying
# more than one sync-wait ("Too many sync wait commands", CoreV3GenImpl
# setupSyncWait), while Tile routinely attaches several waits to one
# instruction. Post-process the BIR JSON before compilation: move extra waits
# onto single-wait NoOps inserted immediately before the instruction on the
# same (in-order) engine sequencer — a strictly stronger ordering, so always
# semantics-preserving.
# ---------------------------------------------------------------------------

_MAX_WAITS = 1


def _split_multi_waits(bir: dict) -> dict:
    ctr = 0
    for fn in bir.get("functions", []):
        for bb in fn.get("blocks", []):
            out = []
            for ins in bb.get("instructions", []):
                si = ins.get("sync_info")
                ow = (si or {}).get("on_wait") or []
                if len(ow) > _MAX_WAITS:
                    for w in ow[: -_MAX_WAITS]:
                        ctr += 1
                        out.append(
                            {
                                "debug": ins.get("debug"),
                                "engine": ins.get("engine"),
                                "ins": [],
                                "name": f"I-WSPLIT-{ctr}",
                                "opcode": "NoOp",
                                "outs": [],
                                "sync_info": {"on_update": [], "on_wait": [w]},
                            }
                        )
                    si["on_wait"] = ow[-_MAX_WAITS:]
                out.append(ins)
            bb["instructions"] = out
    return bir


_orig_compile_bir_kernel = bass_utils.compile_bir_kernel


def _compile_bir_kernel_split(bir_json, tmpdir, neff_name="file.neff"):
    bir = json.loads(bir_json)
    bir = _split_multi_waits(bir)
    return _orig_compile_bir_kernel(json.dumps(bir).encode(), tmpdir, neff_name)


if bass_utils.compile_bir_kernel is not _compile_bir_kernel_split:
    bass_utils.compile_bir_kernel = _compile_bir_kernel_split
    bass2jax.compile_bir_kernel = _compile_bir_kernel_split


def _cheap_drain_and_barrier(self, tick_clock, wait_clock):
    # Cheap kernel tail: stock TileContext runs drain + two all-engine
    # butterfly barriers around the semaphore clear (~8us). Instead, attach
    # every outstanding proc's final tick as waits on GpSimd — the engine
    # that performs the DGE/sem clear. Once those waits pass, every engine
    # is quiescent, so the clear is safe and the other engines simply halt.
    # (The multi-wait NOP is split into single-wait NOPs by the BIR pass.)
    nc = self.nc
    collector = nc.gpsimd.nop(nofuse=True)
    wait_clock.add_sem_waits(
        collector.ins, ScopedClock({None: tick_clock.global_clock})
    )
    nc.sync.drain()
    assert self.sems is not None
    popped = nc._tile_sem_poison_stack.pop()
    assert popped is self._sem_poison
    nc.clear_and_free_semaphores(list(self.sems.allocated().values()))


tile_mod.TileContext._drain_and_barrier = _cheap_drain_and_barrier


def _build_bass(cdt=BF16):
    nc = bass.Bass(trn_type="TRN2")
    # Pre-tiled inputs: every tensor is laid out so one SBUF partition line
    # is one contiguous DRAM run (big DMA descriptors).
    #   xd[e, p, c, t]      = x_sorted[e*CAP + t, c*128 + p]
    #   w1e0[f, p, c, fi]   = W1[core_e0, c*128 + p, f*128 + fi]   (expert 0)
    #   w1d[e, h2, p, c, fh]= W1[e, c*128 + p, h2*F/2 + fh]        (e >= 1)
    #   w2d[e, p, s, h]     = W2[e, s*128 + p, h]
    xd = nc.dram_tensor("xd", [EPC, P, HS, CAP], cdt, kind="ExternalInput")
    w1e0 = nc.dram_tensor("w1e0", [FS, P, HS, P], cdt, kind="ExternalInput")
    w1d = nc.dram_tensor(
        "w1d", [EPC, 2, P, HS, F // 2], cdt, kind="ExternalInput"
    )
    w2d = nc.dram_tensor("w2d", [EPC, P, FS, H], cdt, kind="ExternalInput")
    wt = nc.dram_tensor("wt", [P, TPC // P], FP32, kind="ExternalInput")
    y = nc.dram_tensor("y", [TPC, H], cdt, kind="ExternalOutput")

    with TileContext(nc) as tc:
        with (
            tc.tile_pool(name="e0", bufs=1) as e0pool,
            tc.tile_pool(name="weights", bufs=3) as wpool,
            tc.tile_pool(name="acts", bufs=3) as apool,
            tc.tile_pool(name="outs", bufs=8) as opool,
            tc.tile_pool(name="consts", bufs=1) as cpool,
            tc.tile_pool(name="psum", bufs=4, space="PSUM") as pspool,
        ):
            wt_t = cpool.tile([P, TPC // P], FP32, tag="wt")

            # HAM warm-up: PE runs at 1.2 GHz until ~3.4us of sustained
            # activity. 3 dummy N=512 matmuls on a zeroed scratch tile bridge
            # the gap between the engine preamble (~7.1us) and the first
            # real matmul's data (~8.3us), so the HAM busy window starts
            # accumulating immediately and the clock is warm by ~10.5us.
            # Scratch PSUM uses the stage-2 tag, idle until ~20us.
            warm_t = cpool.tile([P, CAP], cdt, tag="warm")
            nc.gpsimd.memset(warm_t[:], 0)
            warm_ps = pspool.tile([P, H], FP32, tag="ps2")
            for _ in range(3):
                nc.tensor.matmul(
                    warm_ps[:], warm_t[:, :P], warm_t[:], start=True, stop=True
                )

            x_tiles = {}
            hid_tiles = {}
            w2_tiles = {}
            w1_tiles = {}

            # Three HWDGE rings (only SP/ACT/POOL can initiate DMA):
            # x on scalar(ACT), w1 on sync(SP), w2 + y stores on
            # gpsimd(POOL). All three run concurrently.
            def load_xw1(e):
                if e == 0:
                    # Fine-grained first fill: x as 4 c-chunks, w1 as 8
                    # f-group tiles, each 128KB, so matmul (f,c) is gated
                    # only on x chunk c and w1 group f.
                    xs = []
                    for c in range(HS):
                        x_t = e0pool.tile([P, CAP], cdt, tag=f"x0_{c}")
                        nc.scalar.dma_start(x_t[:], xd[0, :, c, :])
                        xs.append(x_t)
                    ws = []
                    for f in range(FS):
                        w_t = e0pool.tile([P, HS, P], cdt, tag=f"w1e0_{f}")
                        nc.sync.dma_start(w_t[:], w1e0[f])
                        ws.append(w_t)
                    x_tiles[e] = xs
                    w1_tiles[e] = ws
                else:
                    x_t = apool.tile([P, HS, CAP], cdt, tag="x")
                    nc.scalar.dma_start(x_t[:], xd[e])
                    x_tiles[e] = x_t
                    # w1 as two tiles split along F: the first FS/2 matmul
                    # groups only need w1a, so stage 1 can start after half
                    # the weight load.
                    w1a_t = wpool.tile([P, HS, F // 2], cdt, tag="w1a")
                    nc.sync.dma_start(w1a_t[:], w1d[e, 0])
                    w1b_t = wpool.tile([P, HS, F // 2], cdt, tag="w1b")
                    nc.sync.dma_start(w1b_t[:], w1d[e, 1])
                    w1_tiles[e] = (w1a_t, w1b_t)

            def load_w2(e):
                w2_t = wpool.tile([P, FS, H], cdt, tag="w2")
                nc.gpsimd.dma_start(w2_t[:], w2d[e])
                w2_tiles[e] = w2_t
                if e == 0:
                    # routing weights aren't needed until the first stage-2
                    # scale; keep them off the critical fill path
                    nc.scalar.dma_start(wt_t[:], wt[:])

            def stage1(e):
                x_src = x_tiles.pop(e)
                w1_src = w1_tiles.pop(e)
                # ---- stage 1: hid[F, tok] = silu(W1^T x) ----
                hid_t = apool.tile([P, FS, CAP], cdt, tag="hid")
                hid_tiles[e] = hid_t
                for f in range(FS):
                    ps1 = pspool.tile([P, CAP], FP32, tag="ps1")
                    for c in range(HS):
                        if e == 0:
                            lhsT = w1_src[f][:, c, :]
                            rhs = x_src[c][:]
                        else:
                            w1h = w1_src[f // (FS // 2)]
                            fh = f % (FS // 2)
                            lhsT = w1h[:, c, fh * P : (fh + 1) * P]
                            rhs = x_src[:, c, :]
                        nc.tensor.matmul(
                            ps1[:],
                            lhsT,
                            rhs,
                            start=(c == 0),
                            stop=(c == HS - 1),
                        )
                    nc.scalar.activation(
                        hid_t[:, f, :], ps1[:], mybir.ActivationFunctionType.Silu
                    )

            def stage2(e):
                # ---- stage 2: y[tok, H] = (hid^T W2) * wt, stored bf16 ----
                hid_t = hid_tiles.pop(e)
                w2_t = w2_tiles.pop(e)
                for j in range(CS):
                    gj = e * CS + j  # global token-chunk index within this core
                    ps2 = pspool.tile([P, H], FP32, tag="ps2")
                    for f in range(FS):
                        nc.tensor.matmul(
                            ps2[:],
                            hid_t[:, f, j * P : (j + 1) * P],
                            w2_t[:, f, :],
                            start=(f == 0),
                            stop=(f == FS - 1),
                        )
                    y_t = opool.tile([P, H], cdt, tag="y")
                    nc.vector.tensor_scalar_mul(
                        y_t[:], ps2[:], wt_t[:, gj : gj + 1]
                    )
                    nc.gpsimd.dma_start(
                        y[e * CAP + j * P : e * CAP + (j + 1) * P, :], y_t[:]
                    )

            # Software pipeline: stage2(e) is issued after stage1(e+1) so the
            # PE never waits on the ACT (silu) tail of its own expert; loads
            # run one expert ahead of compute.
            load_xw1(0)
            for e in range(EPC):
                if e + 1 < EPC:
                    load_xw1(e + 1)
                load_w2(e)
                stage1(e)
                if e > 0:
                    stage2(e - 1)
            stage2(EPC - 1)
    return nc


_NC_CACHE = {}


def _get_bass(cdt):
    if cdt not in _NC_CACHE:
        _NC_CACHE[cdt] = _build_bass(cdt)
    return _NC_CACHE[cdt]


def kernel(hidden_states, expert_weights, expert_ids, W1, W2):
    hidden_states = np.ascontiguousarray(hidden_states, dtype=np.float32)
    expert_weights = np.ascontiguousarray(expert_weights, dtype=np.float32)
    expert_ids = np.ascontiguousarray(expert_ids, dtype=np.int32)
    W1 = np.ascontiguousarray(W1, dtype=np.float32)
    W2 = np.ascontiguousarray(W2, dtype=np.float32)

    # Dispatch: stable sort of flattened (token, slot) assignments by expert
    # id; fixed-capacity groups of CAP rows, exactly as the reference does.
    flat_ids = expert_ids.reshape(-1)
    order = np.argsort(flat_ids, kind="stable")
    tok = order // K
    w_sorted = expert_weights.reshape(-1)[order]

    np_cdt = ml_dtypes.bfloat16
    xg = hidden_states.astype(np_cdt)[tok]  # [N, H] sorted-assignment rows
    W1_c = W1.astype(np_cdt)
    W2_c = W2.astype(np_cdt)

    in_maps = []
    for core in range(NCORES):
        sl = slice(core * TPC, (core + 1) * TPC)
        g0 = core * EPC
        # x: [e, p, c, t] = xg[core_tok + e*CAP + t, c*128 + p]
        xd = np.ascontiguousarray(
            xg[sl].reshape(EPC, CAP, HS, P).transpose(0, 3, 2, 1)
        )
        # w1 (expert 0 of this core): [f, p, c, fi] = W1[g0, c*128+p, f*128+fi]
        w1e0 = np.ascontiguousarray(
            W1_c[g0].reshape(HS, P, FS, P).transpose(2, 1, 0, 3)
        )
        # w1 halves: [e, h2, p, c, fh] = W1[g0+e, c*128+p, h2*512+fh]
        w1d = np.ascontiguousarray(
            W1_c[g0 : g0 + EPC]
            .reshape(EPC, HS, P, 2, F // 2)
            .transpose(0, 3, 2, 1, 4)
        )
        # w2: [e, p, s, h] = W2[g0+e, s*128+p, h]
        w2d = np.ascontiguousarray(
            W2_c[g0 : g0 + EPC].reshape(EPC, FS, P, H).transpose(0, 2, 1, 3)
        )
        in_maps.append(
            {
                "xd": xd,
                "w1e0": w1e0,
                "w1d": w1d,
                "w2d": w2d,
                "wt": np.ascontiguousarray(
                    w_sorted[sl].reshape(TPC // P, P).T
                ),
            }
        )

    nc = _get_bass(BF16)
    res = run_bass_kernel_spmd(nc, in_maps, core_ids=list(range(NCORES)))
    global _LAST_RESULTS
    _LAST_RESULTS = res
    y_all = np.concatenate(
        [r["y"].astype(np.float32) for r in res.results], axis=0
    )  # [N, H]

    # Combine: undo the sort, then sum each token's K weighted slot outputs.
    y_unsorted = np.empty_like(y_all)
    y_unsorted[order] = y_all
    out = y_unsorted.reshape(B, K, H).sum(axis=1)
    return np.ascontiguousarray(out, dtype=np.float32)


# revision 11
# speedup vs baseline: 1.0852x; 1.0852x over previous
"""Grouped MoE dispatcher kernel for 8 Trainium2 NeuronCores.

Expert-parallel: 8 experts per core. Host performs the dispatch (stable sort
of (token, slot) assignments by expert id — identical to the reference's
fixed-capacity grouped dispatch) and supplies each core its 8 experts'
tokens pre-gathered and pre-tiled into SBUF layout; the device runs the
grouped FFN (x@W1 -> silu -> @W2, scaled by routing weight) as bf16 matmuls
with fp32 PSUM accumulation; host scatter-combines the two slots per token.

All DRAM tensors are laid out so every DMA descriptor is one full SBUF
partition line (1-8 KB contiguous), and expert 0's x/W1 are additionally
split into fine-grained chunks so the first matmul starts as soon as
~256 KB have landed. y is stored bf16 (host upcasts when combining).

Problem constants (hardcoded): B=16384 tokens, K=2, E=64 experts, H=512,
F=1024; I/O fp32, matmul operands bf16 (end-to-end rel err ~3.4e-3).
"""

import json
import os

import ml_dtypes
import numpy as np

import concourse.bass as bass
import concourse.bass2jax as bass2jax
import concourse.bass_utils as bass_utils
import concourse.mybir as mybir
import concourse.tile as tile_mod
from concourse.tile import TileContext, ScopedClock
from concourse.bass_utils import run_bass_kernel_spmd

B = 16384
K = 2
E = 64
H = 512
F = 1024
NCORES = 8
EPC = E // NCORES          # experts per core = 8
N = B * K                  # assignments = 32768
CAP = N // E               # per-expert capacity = 512
TPC = EPC * CAP            # tokens (assignments) per core = 4096
P = 128                    # partitions

FP32 = mybir.dt.float32
BF16 = mybir.dt.bfloat16

HS = H // P   # 4 contraction subtiles for stage 1
FS = F // P   # 8 F subtiles (stage-1 out partitions / stage-2 contraction)
CS = CAP // P  # 4 token subtiles per expert


# ---------------------------------------------------------------------------
# Workaround: the walrus build in this container rejects instructions carrying
# more than one sync-wait ("Too many sync wait commands", CoreV3GenImpl
# setupSyncWait), while Tile routinely attaches several waits to one
# instruction. Post-process the BIR JSON before compilation: move extra waits
# onto single-wait NoOps inserted immediately before the instruction on the
# same (in-order) engine sequencer — a strictly stronger ordering, so always
# semantics-preserving.
# ---------------------------------------------------------------------------

_MAX_WAITS = 1


def _split_multi_waits(bir: dict) -> dict:
    ctr = 0
    for fn in bir.get("functions", []):
        for bb in fn.get("blocks", []):
            out = []
            for ins in bb.get("instructions", []):
                si = ins.get("sync_info")
                ow = (si or {}).get("on_wait") or []
                if len(ow) > _MAX_WAITS:
                    for w in ow[: -_MAX_WAITS]:
                        ctr += 1
                        out.append(
                            {
                                "debug": ins.get("debug"),
                                "engine": ins.get("engine"),
                                "ins": [],
                                "name": f"I-WSPLIT-{ctr}",
                                "opcode": "NoOp",
                                "outs": [],
                                "sync_info": {"on_update": [], "on_wait": [w]},
                            }
                        )
                    si["on_wait"] = ow[-_MAX_WAITS:]
                out.append(ins)
            bb["instructions"] = out
    return bir


_orig_compile_bir_kernel = bass_utils.compile_bir_kernel


def _compile_bir_kernel_split(bir_json, tmpdir, neff_name="file.neff"):
    bir = json.loads(bir_json)
    bir = _split_multi_waits(bir)
    return _orig_compile_bir_kernel(json.dumps(bir).encode(), tmpdir, neff_name)


if bass_utils.compile_bir_kernel is not _compile_bir_kernel_split:
    bass_utils.compile_bir_kernel = _compile_bir_kernel_split
    bass2jax.compile_bir_kernel = _compile_bir_kernel_split


def _cheap_drain_and_barrier(self, tick_clock, wait_clock):
    # Cheap kernel tail: stock TileContext runs drain + two all-engine
    # butterfly barriers around the semaphore clear (~8us). Instead, attach
    # every outstanding proc's final tick as waits on GpSimd — the engine
    # that performs the DGE/sem clear. Once those waits pass, every engine
    # is quiescent, so the clear is safe and the other engines simply halt.
    # (The multi-wait NOP is split into single-wait NOPs by the BIR pass.)
    nc = self.nc
    collector = nc.gpsimd.nop(nofuse=True)
    wait_clock.add_sem_waits(
        collector.ins, ScopedClock({None: tick_clock.global_clock})
    )
    nc.sync.drain()
    assert self.sems is not None
    popped = nc._tile_sem_poison_stack.pop()
    assert popped is self._sem_poison
    nc.clear_and_free_semaphores(list(self.sems.allocated().values()))


tile_mod.TileContext._drain_and_barrier = _cheap_drain_and_barrier


def _build_bass(cdt=BF16):
    nc = bass.Bass(trn_type="TRN2")
    # Pre-tiled inputs: every tensor is laid out so one SBUF partition line
    # is one contiguous DRAM run (big DMA descriptors).
    #   xd[e, p, c, t]      = x_sorted[e*CAP + t, c*128 + p]
    #   w1e0[f, p, c, fi]   = W1[core_e0, c*128 + p, f*128 + fi]   (expert 0)
    #   w1d[e, h2, p, c, fh]= W1[e, c*128 + p, h2*F/2 + fh]        (e >= 1)
    #   w2d[e, p, s, h]     = W2[e, s*128 + p, h]
    xd = nc.dram_tensor("xd", [EPC, P, HS, CAP], cdt, kind="ExternalInput")
    w1d = nc.dram_tensor(
        "w1d", [EPC, 2, P, HS, F // 2], cdt, kind="ExternalInput"
    )
    w2d = nc.dram_tensor("w2d", [EPC, P, FS, H], cdt, kind="ExternalInput")
    wt = nc.dram_tensor("wt", [P, TPC // P], FP32, kind="ExternalInput")
    y = nc.dram_tensor("y", [TPC, H], cdt, kind="ExternalOutput")

    with TileContext(nc) as tc:
        with (
            tc.tile_pool(name="weights", bufs=3) as wpool,
            tc.tile_pool(name="acts", bufs=3) as apool,
            tc.tile_pool(name="outs", bufs=8) as opool,
            tc.tile_pool(name="consts", bufs=1) as cpool,
            tc.tile_pool(name="psum", bufs=4, space="PSUM") as pspool,
        ):
            wt_t = cpool.tile([P, TPC // P], FP32, tag="wt")

            # HAM warm-up: PE runs at 1.2 GHz until ~3.4us of sustained
            # activity. 8 dummy N=512 matmuls on a zeroed scratch tile span
            # the window between the engine preamble end (~7.1us) and the
            # arrival of expert 0's x + first W1 half (~10.5us at the
            # ~350 GB/s HBM cap), so the real matmuls start at 2.4 GHz.
            # Scratch PSUM uses the stage-2 tag, idle until ~24us.
            warm_t = cpool.tile([P, CAP], cdt, tag="warm")
            nc.gpsimd.memset(warm_t[:], 0)
            warm_ps = pspool.tile([P, H], FP32, tag="ps2")
            for _ in range(8):
                nc.tensor.matmul(
                    warm_ps[:], warm_t[:, :P], warm_t[:], start=True, stop=True
                )

            x_tiles = {}
            hid_tiles = {}
            w2_tiles = {}
            w1_tiles = {}

            # Three HWDGE rings (only SP/ACT/POOL can initiate DMA). Each
            # ring drains through ONE hw queue; queues arbitrate HBM
            # per-packet, and the early fill runs at the ~350 GB/s HBM cap,
            # so ring assignment is a bandwidth-priority decision:
            #   scalar(ACT): x + wt  — 4KB packets
            #   sync(SP):    w1 then w2, strictly in need-order — 4/8KB
            #   gpsimd(POOL): y stores only (bf16, light, never competes
            #                 with the input fill)
            def load_xw1(e):
                x_t = apool.tile([P, HS, CAP], cdt, tag="x")
                nc.scalar.dma_start(x_t[:], xd[e])
                x_tiles[e] = x_t
                # w1 as two tiles split along F: the first FS/2 matmul
                # groups only need w1a, so stage 1 can start after half
                # the weight load.
                w1a_t = wpool.tile([P, HS, F // 2], cdt, tag="w1a")
                nc.sync.dma_start(w1a_t[:], w1d[e, 0])
                w1b_t = wpool.tile([P, HS, F // 2], cdt, tag="w1b")
                nc.sync.dma_start(w1b_t[:], w1d[e, 1])
                w1_tiles[e] = (w1a_t, w1b_t)

            def load_w2(e):
                # issued after load_xw1(e+1): on the shared sync queue this
                # 1MB transfer must never delay the next expert's stage-1
                # weights (the early fill is HBM-bound)
                w2_t = wpool.tile([P, FS, H], cdt, tag="w2")
                nc.sync.dma_start(w2_t[:], w2d[e])
                w2_tiles[e] = w2_t
                if e == 0:
                    # routing weights aren't needed until the first stage-2
                    # scale; keep them off the critical fill path
                    nc.scalar.dma_start(wt_t[:], wt[:])

            def stage1(e):
                x_src = x_tiles.pop(e)
                w1_src = w1_tiles.pop(e)
                # ---- stage 1: hid[F, tok] = silu(W1^T x) ----
                hid_t = apool.tile([P, FS, CAP], cdt, tag="hid")
                hid_tiles[e] = hid_t
                for f in range(FS):
                    ps1 = pspool.tile([P, CAP], FP32, tag="ps1")
                    for c in range(HS):
                        w1h = w1_src[f // (FS // 2)]
                        fh = f % (FS // 2)
                        nc.tensor.matmul(
                            ps1[:],
                            w1h[:, c, fh * P : (fh + 1) * P],
                            x_src[:, c, :],
                            start=(c == 0),
                            stop=(c == HS - 1),
                        )
                    nc.scalar.activation(
                        hid_t[:, f, :], ps1[:], mybir.ActivationFunctionType.Silu
                    )

            def stage2(e):
                # ---- stage 2: y[tok, H] = (hid^T W2) * wt, stored bf16 ----
                hid_t = hid_tiles.pop(e)
                w2_t = w2_tiles.pop(e)
                for j in range(CS):
                    gj = e * CS + j  # global token-chunk index within this core
                    ps2 = pspool.tile([P, H], FP32, tag="ps2")
                    for f in range(FS):
                        nc.tensor.matmul(
                            ps2[:],
                            hid_t[:, f, j * P : (j + 1) * P],
                            w2_t[:, f, :],
                            start=(f == 0),
                            stop=(f == FS - 1),
                        )
                    y_t = opool.tile([P, H], cdt, tag="y")
                    nc.vector.tensor_scalar_mul(
                        y_t[:], ps2[:], wt_t[:, gj : gj + 1]
                    )
                    nc.gpsimd.dma_start(
                        y[e * CAP + j * P : e * CAP + (j + 1) * P, :], y_t[:]
                    )

            # Software pipeline: stage2(e) is issued after stage1(e+1) so the
            # PE never waits on the ACT (silu) tail of its own expert; loads
            # run one expert ahead of compute.
            load_xw1(0)
            for e in range(EPC):
                if e + 1 < EPC:
                    load_xw1(e + 1)
                load_w2(e)
                stage1(e)
                if e > 0:
                    stage2(e - 1)
            stage2(EPC - 1)
    return nc


_NC_CACHE = {}


def _get_bass(cdt):
    if cdt not in _NC_CACHE:
        _NC_CACHE[cdt] = _build_bass(cdt)
    return _NC_CACHE[cdt]


def kernel(hidden_states, expert_weights, expert_ids, W1, W2):
    hidden_states = np.ascontiguousarray(hidden_states, dtype=np.float32)
    expert_weights = np.ascontiguousarray(expert_weights, dtype=np.float32)
    expert_ids = np.ascontiguousarray(expert_ids, dtype=np.int32)
    W1 = np.ascontiguousarray(W1, dtype=np.float32)
    W2 = np.ascontiguousarray(W2, dtype=np.float32)

    # Dispatch: stable sort of flattened (token, slot) assignments by expert
    # id; fixed-capacity groups of CAP rows, exactly as the reference does.
    flat_ids = expert_ids.reshape(-1)
    order = np.argsort(flat_ids, kind="stable")
    tok = order // K
    w_sorted = expert_weights.reshape(-1)[order]

    np_cdt = ml_dtypes.bfloat16
    xg = hidden_states.astype(np_cdt)[tok]  # [N, H] sorted-assignment rows
    W1_c = W1.astype(np_cdt)
    W2_c = W2.astype(np_cdt)

    in_maps = []
    for core in range(NCORES):
        sl = slice(core * TPC, (core + 1) * TPC)
        g0 = core * EPC
        # x: [e, p, c, t] = xg[core_tok + e*CAP + t, c*128 + p]
        xd = np.ascontiguousarray(
            xg[sl].reshape(EPC, CAP, HS, P).transpose(0, 3, 2, 1)
        )
        # w1 halves: [e, h2, p, c, fh] = W1[g0+e, c*128+p, h2*512+fh]
        w1d = np.ascontiguousarray(
            W1_c[g0 : g0 + EPC]
            .reshape(EPC, HS, P, 2, F // 2)
            .transpose(0, 3, 2, 1, 4)
        )
        # w2: [e, p, s, h] = W2[g0+e, s*128+p, h]
        w2d = np.ascontiguousarray(
            W2_c[g0 : g0 + EPC].reshape(EPC, FS, P, H).transpose(0, 2, 1, 3)
        )
        in_maps.append(
            {
                "xd": xd,
                "w1d": w1d,
                "w2d": w2d,
                "wt": np.ascontiguousarray(
                    w_sorted[sl].reshape(TPC // P, P).T
                ),
            }
        )

    nc = _get_bass(BF16)
    res = run_bass_kernel_spmd(nc, in_maps, core_ids=list(range(NCORES)))
    global _LAST_RESULTS
    _LAST_RESULTS = res
    y_all = np.concatenate(
        [r["y"].astype(np.float32) for r in res.results], axis=0
    )  # [N, H]

    # Combine: undo the sort, then sum each token's K weighted slot outputs.
    y_unsorted = np.empty_like(y_all)
    y_unsorted[order] = y_all
    out = y_unsorted.reshape(B, K, H).sum(axis=1)
    return np.ascontiguousarray(out, dtype=np.float32)
